# revision 58
# baseline (speedup 1.0000x reference)
"""Multi-head causal attention (B=2, S=2048, D=1024, H=16) on 8 trn2 NeuronCores.

Sharding: core c handles batch b = c//4 and head group g = c%4 (heads 4g..4g+3).
Each core computes:
  qkv projection for its 4 heads        [2048,1024] @ [1024,3*256]
  causal attention for its 4 heads      (scoresT layout, softmax w/o max-sub,
                                         causality exploited at 128 blocks)
  partial output projection             ctx_c @ w_out[rows] -> [2048,1024]
Host sums the 4 bf16 partial outputs per batch in fp32.

Matmuls run in bf16; accumulation is fp32 in PSUM. The softmax denominator
comes free from a ones-column appended to v. PSUM tiles are paired
([128,1024] = 2 banks) so activation/copy drains are fewer+wider, and the
attention stream is software-pipelined (QK of head h overlaps AV of head
h-1, out-proj of block qb-1 overlaps QK of head 0) to keep the PE gapless.
"""

import sys
from contextlib import ExitStack

for _p in ("/opt/trn_rl_repo",):
    if _p not in sys.path:
        sys.path.insert(0, _p)

import numpy as np

import concourse.bass as bass  # noqa: F401
import concourse.tile as tile
from concourse import bacc, bass_utils, mybir

B, S, D, H, HD = 2, 2048, 1024, 16, 64
P = 128
NCORES = 8
NT = S // P          # 16 token tiles
KD = D // P          # 8 contraction tiles over D
NB = S // 512        # 4 query blocks of 512
HPC = 4              # heads per core
WCOLS = HPC * HD     # 256 weight columns per core per q/k/v

F32 = mybir.dt.float32
BF16 = mybir.dt.bfloat16
EXP = mybir.ActivationFunctionType.Exp

DT = BF16
VW = 128             # v1 block width (ones col + 63 pad + 64 value cols)


def prep(x: np.ndarray) -> np.ndarray:
    import ml_dtypes

    return np.ascontiguousarray(x, np.float32).astype(ml_dtypes.bfloat16)


def _emit(tc: tile.TileContext, aps: dict):
    nc = tc.nc
    xT, wq, wk, wv, wo, tri, out = (
        aps["xT"], aps["wq"], aps["wk"], aps["wv"], aps["wo"],
        aps["tri"], aps["out"],
    )

    with ExitStack() as top:
        qk_pool = top.enter_context(tc.tile_pool(name="qk", bufs=1))
        ctx_pool = top.enter_context(tc.tile_pool(name="ctxT", bufs=1))
        wo_pool = top.enter_context(tc.tile_pool(name="wo", bufs=2))
        const_pool = top.enter_context(tc.tile_pool(name="const", bufs=1))
        small_pool = top.enter_context(tc.tile_pool(name="small", bufs=4))
        out_pool = top.enter_context(tc.tile_pool(name="outsb", bufs=3))
        exp_pool = top.enter_context(tc.tile_pool(name="expT", bufs=20))
        x_pool = top.enter_context(tc.tile_pool(name="xc", bufs=NB))
        w_pool = top.enter_context(tc.tile_pool(name="w", bufs=1))
        ps2 = top.enter_context(tc.tile_pool(name="ps2", bufs=2, space="PSUM"))
        ctxps_pool = top.enter_context(
            tc.tile_pool(name="ctxps", bufs=4, space="PSUM")
        )

        # persistent SBUF tiles; p-index (head pair) merged into the free axis
        qTm = qk_pool.tile([P, 2 * S], DT, tag="qT")
        kTm = qk_pool.tile([P, 2 * S], DT, tag="kT")
        ctxTm = ctx_pool.tile([P, 2 * S], DT, tag="ctxT")
        v1all = const_pool.tile([P, NT * HPC * VW], DT, tag="v1")
        wo_sb = [wo_pool.tile([P, D], DT, tag="wo", name=f"wo{i}") for i in range(2)]
        tri_sb = const_pool.tile([P, P], DT, tag="tri")
        ones64 = const_pool.tile([P, NT * HPC], F32, tag="ones64")
        nc.vector.memset(ones64[:], 1.0)
        # v1 blocks are 128 wide: ones column at 0 (denominator lands on PSUM
        # partition 0 where reciprocal_approx_fast can read it directly),
        # cols 1..63 unused garbage, value columns at 64..127 (64-aligned
        # reads for the normalize multiply). PE time is moving-bound, so the
        # wider stationary costs nothing.
        v1v = v1all[:].rearrange("p (t a c) -> p t a c", a=HPC, c=VW)
        nc.scalar.copy(
            v1v[:, :, :, 0:1],
            ones64[:].rearrange("p (t a c) -> p t a c", t=NT, c=1),
        )

        wqm = w_pool.tile([P, KD * WCOLS], DT, tag="wq")
        wkm = w_pool.tile([P, KD * WCOLS], DT, tag="wk")
        wvm = w_pool.tile([P, KD * WCOLS], DT, tag="wv")
        # Consolidated input DMAs (few big descriptors; order matches phase-1
        # consumption so the q-chain of nb=0 can start as early as possible).
        xcall = [
            x_pool.tile([P, KD * 512], DT, tag="xc", name=f"xcall{nb}")
            for nb in range(NB)
        ]

        def xc(kt, nb, c0=0, c1=512):
            return xcall[nb][:, kt * 512 + c0 : kt * 512 + c1]

        def dma_w(dstm, src, k0=0, k1=KD):
            nc.sync.dma_start(
                dstm[:, k0 * WCOLS : k1 * WCOLS].rearrange(
                    "p (k c) -> p k c", c=WCOLS
                ),
                src.rearrange("(k p) c -> p k c", p=P)[:, k0:k1, :],
            )

        def dma_x(nb, k0=0, k1=KD):
            nc.sync.dma_start(
                xcall[nb][:, k0 * 512 : k1 * 512].rearrange(
                    "p (k c) -> p k c", c=512
                ),
                xT[:].rearrange("(k p) c -> p k c", p=P)[
                    :, k0:k1, nb * 512 : (nb + 1) * 512
                ],
            )

        # first q-chain dependencies split fine so compute starts early
        for k0 in range(0, KD, 2):
            dma_w(wqm, wq, k0, k0 + 2)
            dma_x(0, k0, k0 + 2)
        dma_w(wkm, wk)
        dma_w(wvm, wv)
        nc.sync.dma_start(tri_sb[:], tri[:])
        for nb in range(1, NB):
            dma_x(nb)
        for i in range(2):
            nc.sync.dma_start(wo_sb[i][:], wo[i * P : (i + 1) * P, :])

        # ===== Phase 1: qkv projection (nb-major, paired psum) ===============
        def emit_proj_qk(nb, wm, dstm):
            psqk = ps2.tile([P, 1024], F32, tag="ps2")
            for p2 in range(2):
                for kt in range(KD):
                    nc.tensor.matmul(
                        psqk[:, p2 * 512 : (p2 + 1) * 512],
                        wm[:, kt * WCOLS + p2 * P : kt * WCOLS + (p2 + 1) * P],
                        xc(kt, nb),
                        start=(kt == 0),
                        stop=(kt == KD - 1),
                    )
            # one paired drain: halves go to free-blocks 0 / 1 of dstm
            dview = dstm[:].rearrange("p (a c) -> p a c", c=S)
            nc.scalar.copy(
                dview[:, :, nb * 512 : (nb + 1) * 512],
                psqk[:].rearrange("p (a c) -> p a c", c=512),
            )

        def emit_proj_v(nb):
            psv = ps2.tile([P, 1024], F32, tag="ps2")
            for tloc in range(4):
                for kt in range(KD):
                    nc.tensor.matmul(
                        psv[:, tloc * WCOLS : (tloc + 1) * WCOLS],
                        xc(kt, nb, tloc * P, (tloc + 1) * P),
                        wvm[:, kt * WCOLS : (kt + 1) * WCOLS],
                        start=(kt == 0),
                        stop=(kt == KD - 1),
                    )
            # paired v drains (ACT is idle in phase 1): value cols at 64..127
            for pr in range(2):
                t0 = nb * 4 + 2 * pr
                nc.scalar.copy(
                    v1v[:, t0 : t0 + 2, :, HD:VW],
                    psv[:, 2 * pr * WCOLS : (2 * pr + 2) * WCOLS].rearrange(
                        "p (t a c) -> p t a c", t=2, c=HD
                    ),
                )

        # ===== Phase 2+3: attention, software-pipelined ======================
        def v1_stat(jt, h):
            base = (jt * HPC + h) * VW
            return v1all[:, base : base + VW]

        def emit_qk(qb, h, pr, ets):
            """QK pair pr for head h of block qb + exp (+tri on diagonal)."""
            p, off = h // 2, 64 * (h % 2)
            pbase = p * S
            q0 = qb * 512
            jta, jtb = 2 * pr, 2 * pr + 1
            pss = ps2.tile([P, 1024], F32, tag="ps2")
            et = exp_pool.tile([P, 1024], DT, tag="expT")
            for half, jt in ((0, jta), (1, jtb)):
                m = jt - 4 * qb
                lo = P * m if m > 0 else 0
                base = 512 * half
                nc.tensor.matmul(
                    pss[:, base + lo : base + 512],
                    kTm[off : off + 64, pbase + jt * P : pbase + (jt + 1) * P],
                    qTm[off : off + 64, pbase + q0 + lo : pbase + q0 + 512],
                    start=True,
                    stop=True,
                )
            if jta >= 4 * qb:  # diagonal pair: separate exps + triangle masks
                for half, jt in ((0, jta), (1, jtb)):
                    m = jt - 4 * qb
                    lo = P * m if m > 0 else 0
                    base = 512 * half
                    nc.scalar.activation(
                        et[:, base + lo : base + 512], pss[:, base + lo : base + 512], EXP
                    )
                    nc.vector.tensor_mul(
                        et[:, base + lo : base + lo + P],
                        et[:, base + lo : base + lo + P],
                        tri_sb[:],
                    )
            else:
                nc.scalar.activation(et[:], pss[:], EXP)
            ets[(h, pr)] = et

        def emit_av(qb, h, pr, npairs, ets, ctxps):
            """AV pair pr for head h accumulating into ctxps."""
            et = ets[(h, pr)]
            njt = 2 * npairs
            for half, jt in ((0, 2 * pr), (1, 2 * pr + 1)):
                m = jt - 4 * qb
                lo = P * m if m > 0 else 0
                base = 512 * half
                nc.tensor.matmul(
                    ctxps[:, lo:512],
                    v1_stat(jt, h),
                    et[:, base + lo : base + 512],
                    start=(jt == 0),
                    stop=(jt == njt - 1),
                )

        def emit_norm_head(qb, h, ctxps, chunks=1):
            """Per-head normalize (chunked for the tail-critical last head)."""
            p, off = h // 2, 64 * (h % 2)
            q0 = qb * 512
            cw = 512 // chunks
            segs = []
            for ck in range(chunks):
                c0, c1 = ck * cw, (ck + 1) * cw
                rec_s = small_pool.tile([1, 512], F32, tag="rec_s")
                nc.vector.reciprocal_approx_fast(
                    out=rec_s[:, c0:c1], in_=ctxps[0:1, c0:c1]
                )
                recb = small_pool.tile([64, 512], F32, tag="recb")
                nc.gpsimd.partition_broadcast(
                    recb[:, c0:c1], rec_s[:, c0:c1], channels=64
                )
                segs.append((recb, c0, c1))
            for recb, c0, c1 in segs:
                nc.vector.tensor_mul(
                    ctxTm[off : off + 64, p * S + q0 + c0 : p * S + q0 + c1],
                    ctxps[64:128, c0:c1],
                    recb[:, c0:c1],
                )

        def emit_outproj_tt(qb, tloc, last=False, eng="act"):
            tt = qb * 4 + tloc
            pso = ps2.tile([P, 1024], F32, tag="ps2", name=f"pso{tt}")
            for ob in range(2):
                for kt2 in range(2):
                    nc.tensor.matmul(
                        pso[:, ob * 512 : (ob + 1) * 512],
                        ctxTm[:, kt2 * S + tt * P : kt2 * S + (tt + 1) * P],
                        wo_sb[kt2][:, ob * 512 : (ob + 1) * 512],
                        start=(kt2 == 0),
                        stop=(kt2 == 1),
                    )
            osb = out_pool.tile([P, D], DT, tag="osb", name=f"osb{tt}")
            if last:
                # split halves across ACT and DVE so the tail drains fast
                nc.scalar.copy(osb[:, 0:512], pso[:, 0:512])
                nc.vector.tensor_copy(osb[:, 512:1024], pso[:, 512:1024])
                nc.sync.dma_start(out[tt * P : (tt + 1) * P, 0:512], osb[:, 0:512])
                nc.sync.dma_start(
                    out[tt * P : (tt + 1) * P, 512:1024], osb[:, 512:1024]
                )
            else:
                # drain on whichever engine has slack in the surrounding
                # stream (ACT during zipped QK+AV, DVE during QK bursts)
                if eng == "dve":
                    nc.vector.tensor_copy(osb[:], pso[:])
                else:
                    nc.scalar.copy(osb[:], pso[:])
                nc.sync.dma_start(out[tt * P : (tt + 1) * P, :], osb[:])

        # Phase 1 with qb0's QK pairs riding in it: the projection chains keep
        # the PE busy while the (ACT-paced) qb0 exps drain in parallel —
        # qb0's QK burst would otherwise idle the PE at attention start.
        qb0_ets = {}
        qb0_qk = [(h, pr) for h in range(HPC) for pr in range(2)]
        emit_proj_qk(0, wqm, qTm)
        emit_proj_qk(0, wkm, kTm)
        emit_proj_v(0)
        qi = 0
        for nb in range(1, NB):
            for chain in (
                lambda: emit_proj_qk(nb, wqm, qTm),
                lambda: emit_proj_qk(nb, wkm, kTm),
                lambda: emit_proj_v(nb),
            ):
                chain()
                if qi < len(qb0_qk):
                    h, pr = qb0_qk[qi]
                    emit_qk(0, h, pr, qb0_ets)
                    qi += 1

        for qb in range(NB):
            npairs = 2 * qb + 2
            ets = qb0_ets if qb == 0 else {}
            last_qb = qb == NB - 1
            ctxps_of = {
                h: ctxps_pool.tile([P, 512], F32, tag="ctxps", name=f"cps{qb}_{h}")
                for h in range(HPC)
            }
            if npairs <= 4:
                # short blocks: group all QKs then all AVs (the per-head zip
                # is too shallow to hide the exp latency here)
                for h in range(HPC):
                    if qb > 0:
                        for pr in range(npairs):
                            emit_qk(qb, h, pr, ets)
                        if h < 2:
                            emit_outproj_tt(qb - 1, 2 * h, eng="dve")
                            emit_outproj_tt(qb - 1, 2 * h + 1)
                for h in range(HPC):
                    for pr in range(npairs):
                        emit_av(qb, h, pr, npairs, ets, ctxps_of[h])
                    emit_norm_head(qb, h, ctxps_of[h])
            else:
                # head 0 QK, interleaved with the previous block's out-proj
                for pr in range(npairs):
                    emit_qk(qb, 0, pr, ets)
                    if pr % 2 == 1 and pr // 2 < 4:
                        emit_outproj_tt(qb - 1, pr // 2)
                for tloc in range(min(npairs // 2, 4), 4):
                    emit_outproj_tt(qb - 1, tloc)
                # heads 1..3 QK, zipped with AV of the previous head
                for h in range(1, HPC):
                    for pr in range(npairs):
                        emit_qk(qb, h, pr, ets)
                        emit_av(qb, h - 1, pr, npairs, ets, ctxps_of[h - 1])
                    emit_norm_head(qb, h - 1, ctxps_of[h - 1])
                for pr in range(npairs):
                    emit_av(qb, HPC - 1, pr, npairs, ets, ctxps_of[HPC - 1])
                emit_norm_head(
                    qb, HPC - 1, ctxps_of[HPC - 1], chunks=4 if last_qb else 1
                )
        for tloc in range(4):
            emit_outproj_tt(NB - 1, tloc, last=True)


_BUILD_CACHE = {}


def build():
    if "nc" in _BUILD_CACHE:
        return _BUILD_CACHE["nc"]
    nc = bacc.Bacc("TRN2", target_bir_lowering=False, debug=False)
    aps = {
        "xT": nc.dram_tensor("xT", [D, S], DT, kind="ExternalInput").ap(),
        "wq": nc.dram_tensor("wq", [D, WCOLS], DT, kind="ExternalInput").ap(),
        "wk": nc.dram_tensor("wk", [D, WCOLS], DT, kind="ExternalInput").ap(),
        "wv": nc.dram_tensor("wv", [D, WCOLS], DT, kind="ExternalInput").ap(),
        "wo": nc.dram_tensor("wo", [WCOLS, D], DT, kind="ExternalInput").ap(),
        "tri": nc.dram_tensor("tri", [P, P], DT, kind="ExternalInput").ap(),
        "out": nc.dram_tensor("out", [S, D], DT, kind="ExternalOutput").ap(),
    }
    with tile.TileContext(nc) as tc:
        _emit(tc, aps)
    nc.compile()
    _BUILD_CACHE["nc"] = nc
    return nc


def make_tri() -> np.ndarray:
    """tri[dj, t] = 1 if dj <= t else 0 (causal keep within a 128 block)."""
    dj = np.arange(P)[:, None]
    t = np.arange(P)[None, :]
    return prep(np.where(dj <= t, 1.0, 0.0).astype(np.float32))


def make_in_maps(x, w_qkv, w_out):
    tri = make_tri()
    in_maps = []
    scale = np.float32(1.0 / np.sqrt(HD))  # 1/8: exact in bf16
    for c in range(NCORES):
        b, g = c // 4, c % 4
        cs = slice(g * WCOLS, (g + 1) * WCOLS)
        in_maps.append(
            {
                "xT": prep(x[b].T),
                "wq": prep(w_qkv[:, g * WCOLS : (g + 1) * WCOLS] * scale),
                "wk": prep(w_qkv[:, D + g * WCOLS : D + (g + 1) * WCOLS]),
                "wv": prep(w_qkv[:, 2 * D + g * WCOLS : 2 * D + (g + 1) * WCOLS]),
                "wo": prep(w_out[cs, :]),
                "tri": tri,
            }
        )
    return in_maps


def kernel(x, w_qkv, w_out, _trace=False):
    nc = build()
    in_maps = make_in_maps(
        np.asarray(x, np.float32), np.asarray(w_qkv, np.float32),
        np.asarray(w_out, np.float32),
    )
    res = bass_utils.run_bass_kernel_spmd(
        nc, in_maps, core_ids=list(range(NCORES)), trace=_trace
    )
    outs = [np.asarray(res.results[c]["out"], dtype=np.float32) for c in range(NCORES)]
    full = np.stack(
        [sum(outs[b * 4 : (b + 1) * 4][1:], outs[b * 4]) for b in range(B)], axis=0
    )
    if _trace:
        kernel.last_results = res
    return full.astype(np.float32)
